# revision 1
# baseline (speedup 1.0000x reference)
"""Distributed attention-head kernel for 8 TRN2 NeuronCores.

Problem: B=4, S=4096, D=1024, H=64
  qs = LN(xs @ Wq); ks = LN(xs @ Wk); vs = xs @ Wv
  out = softmax(qs ks^T / 8) vs          (per batch, full attention)

Sharding: 2 cores per batch element; each core computes the full K/V of its
batch (redundantly, cheap) and attention for its own half of the queries
(2048 rows). No collectives.

Key tricks:
  * Host pre-transposes xs -> xst [D, S] (bf16), with the core's own query
    half moved to the front columns, so one SPMD graph serves all cores.
  * LayerNorm is folded algebraically:
      LN(q).LN(k) = (q.k - 64 mu_q mu_k) / (sig_q sig_k)
    mu rows come free as extra projection columns (8*mean(W, axis=1));
    normalized qn = (q_raw - mu)*rsig is built columnwise with rank-1
    broadcast matmuls (row-select in the lhsT), so scores = kn^T qn with
    contraction exactly 64. Stats for the first half of the sequence are
    processed while the second half is still projecting.
  * Scores are computed transposed (S^T[k, q], k on partitions), each
    128-k-tile as two concurrent column-group matmuls (M=64 each).
  * The softmax denominator folds into the PV matmul via a ones-column in
    V' (lhsT = [V | 1], M=65); the final divide happens on the host.
  * exp() runs on the scalar engine straight out of PSUM; rsqrt is
    exp(-0.5*ln(var+eps)) so only one ACT table set is ever loaded.
"""

import numpy as np
import ml_dtypes

S = 4096
D = 1024
H = 64
HQ = 2048  # queries owned per core
NB = S // 512  # 8 s-blocks of 512
DT = D // 128  # 8 d-tiles
NKT = S // 128  # 32 k-tiles
BF16 = ml_dtypes.bfloat16

_CACHE = {}


def _build_nc():
    import concourse.bacc as bacc
    import concourse.mybir as mybir
    import concourse.tile as tile

    f32 = mybir.dt.float32
    bf16 = mybir.dt.bfloat16
    EXP = mybir.ActivationFunctionType.Exp
    LN_ = mybir.ActivationFunctionType.Ln

    nc = bacc.Bacc("TRN2", target_bir_lowering=False, debug=False, num_devices=8)

    xst_d = nc.dram_tensor("xst", [D, S], bf16, kind="ExternalInput")
    wa_d = nc.dram_tensor("wa", [D, 128], bf16, kind="ExternalInput")
    wb_d = nc.dram_tensor("wb", [D, 66], bf16, kind="ExternalInput")
    sel_d = nc.dram_tensor("sel", [128, 2], bf16, kind="ExternalInput")
    selq_d = nc.dram_tensor("selq", [2, 64], bf16, kind="ExternalInput")
    selk_d = nc.dram_tensor("selk", [2, 64], bf16, kind="ExternalInput")
    selqm_d = nc.dram_tensor("selqm", [2, 64], bf16, kind="ExternalInput")
    selkm_d = nc.dram_tensor("selkm", [2, 64], bf16, kind="ExternalInput")
    ident_d = nc.dram_tensor("ident", [64, 64], bf16, kind="ExternalInput")
    outT_d = nc.dram_tensor("outT", [65, HQ], f32, kind="ExternalOutput")

    with tile.TileContext(nc) as tc:
        with (
            tc.tile_pool(name="const", bufs=1) as cpool,
            tc.tile_pool(name="big", bufs=1) as big,
        ):
          with (
            tc.tile_pool(name="sq", bufs=2) as sqpool,
            tc.tile_pool(name="xs", bufs=3) as xpool,
            tc.tile_pool(name="psA", bufs=2, space="PSUM") as psA_pool,
            tc.tile_pool(name="psB", bufs=2, space="PSUM") as psB_pool,
            tc.tile_pool(name="psMisc", bufs=1, space="PSUM") as psMisc_pool,
            tc.tile_pool(name="psR", bufs=1, space="PSUM") as psR_pool,
          ):
            # constants
            wa_sb = cpool.tile([128, DT, 128], bf16)
            wb_sb = cpool.tile([128, DT, 66], bf16)
            sel_sb = cpool.tile([128, 2], bf16)
            selq_sb = cpool.tile([2, 64], bf16)
            selk_sb = cpool.tile([2, 64], bf16)
            selqm_sb = cpool.tile([2, 64], bf16)
            selkm_sb = cpool.tile([2, 64], bf16)
            ident_sb = cpool.tile([64, 64], bf16)
            zero_sb = cpool.tile([128, 1], f32)
            eps_sb = cpool.tile([16, 1], f32)
            nc.vector.memset(zero_sb[:], 0.0)
            nc.vector.memset(eps_sb[:], 1e-5)
            nc.gpsimd.dma_start(out=wa_sb[:], in_=wa_d.ap().rearrange("(t p) m -> p t m", p=128))
            nc.gpsimd.dma_start(out=wb_sb[:], in_=wb_d.ap().rearrange("(t p) m -> p t m", p=128))
            nc.gpsimd.dma_start(out=sel_sb[:], in_=sel_d[:])
            nc.gpsimd.dma_start(out=selq_sb[:], in_=selq_d[:])
            nc.gpsimd.dma_start(out=selk_sb[:], in_=selk_d[:])
            nc.gpsimd.dma_start(out=selqm_sb[:], in_=selqm_d[:])
            nc.gpsimd.dma_start(out=selkm_sb[:], in_=selkm_d[:])
            nc.gpsimd.dma_start(out=ident_sb[:], in_=ident_d[:])

            # big persistent buffers
            raws = big.tile([128, NB, 512], f32)  # rows 0-63 Q^T_raw, 64-127 K^T_raw
            vt_sb = big.tile([64, S], bf16)       # V^T staging
            qt = big.tile([64, HQ], bf16)         # normalized Q^T
            kt = big.tile([64, S], bf16)          # normalized K^T
            vp = big.tile([128, NKT, 65], bf16)   # V' = [V | ones]
            mu_sb = big.tile([2, S], f32)         # row 0 = 8mu_q, row 1 = -8mu_k
            mu_bf = big.tile([2, S], bf16)
            stats_sb = big.tile([2, S], f32)      # row 0 = sumsq_q, row 1 = sumsq_k
            musq_sb = big.tile([2, S], f32)
            var_sb = big.tile([2, S], f32)
            lnv_sb = big.tile([2, S], f32)
            rsig_bf = big.tile([2, S], bf16)      # row 0 = rq, row 1 = rk
            munorm_bf = big.tile([2, S], bf16)    # mu * rsig (rows as mu_sb)

            xst_r = xst_d.ap().rearrange("(t p) s -> p t s", p=128)

            def stats_and_normalize(h0, h1):
                """Process columns [h0*512, h1*512): var -> rsig -> normalize."""
                cs = slice(h0 * 512, h1 * 512)
                nc.vector.tensor_mul(musq_sb[:, cs], mu_sb[:, cs], mu_sb[:, cs])
                nc.vector.tensor_sub(var_sb[:, cs], stats_sb[:, cs], musq_sb[:, cs])
                nc.scalar.activation(lnv_sb[:, cs], var_sb[:, cs], LN_,
                                     bias=eps_sb[0:2], scale=1.0 / 64.0)
                nc.scalar.activation(rsig_bf[:, cs], lnv_sb[:, cs], EXP,
                                     bias=zero_sb[0:2], scale=-0.5)
                nc.vector.tensor_mul(munorm_bf[:, cs], mu_bf[:, cs], rsig_bf[:, cs])
                for j in range(h0, h1):
                    blk = slice(j * 512, (j + 1) * 512)
                    psRk = psR_pool.tile([64, 512], f32, tag="psr")
                    psMk = psR_pool.tile([64, 512], f32, tag="psm")
                    nc.tensor.matmul(psRk[:], selk_sb[:], rsig_bf[0:2, blk],
                                     start=True, stop=True)
                    nc.tensor.matmul(psMk[:], selkm_sb[:], munorm_bf[0:2, blk],
                                     start=True, stop=True)
                    nc.vector.tensor_mul(kt[:, blk], raws[64:128, j, :], psRk[:])
                    nc.vector.tensor_sub(kt[:, blk], kt[:, blk], psMk[:])
                    if j < 4:
                        psRq = psR_pool.tile([64, 512], f32, tag="psr")
                        psMq = psR_pool.tile([64, 512], f32, tag="psm")
                        nc.tensor.matmul(psRq[:], selq_sb[:], rsig_bf[0:2, blk],
                                         start=True, stop=True)
                        nc.tensor.matmul(psMq[:], selqm_sb[:], munorm_bf[0:2, blk],
                                         start=True, stop=True)
                        nc.vector.tensor_mul(qt[:, blk], raws[0:64, j, :], psRq[:])
                        nc.vector.tensor_sub(qt[:, blk], qt[:, blk], psMq[:])

            # ---------------- phase 1: projections + stats ----------------
            for j in range(NB):
                blk = slice(j * 512, (j + 1) * 512)
                xst_j = xpool.tile([128, DT, 512], bf16, tag="xst")
                if j == 0:
                    # split the first block per d-tile so the PE starts ASAP
                    for t in range(DT):
                        nc.sync.dma_start(out=xst_j[:, t, :], in_=xst_r[:, t, blk])
                else:
                    nc.sync.dma_start(out=xst_j[:], in_=xst_r[:, :, blk])
                psA = psA_pool.tile([128, 512], f32)
                psB = psB_pool.tile([66, 512], f32)
                for t in range(DT):
                    nc.tensor.matmul(psA[:], wa_sb[:, t], xst_j[:, t, :],
                                     start=(t == 0), stop=(t == DT - 1))
                for t in range(DT):
                    nc.tensor.matmul(psB[:], wb_sb[:, t], xst_j[:, t, :],
                                     start=(t == 0), stop=(t == DT - 1))
                # copies out of PSUM (explicit DVE to keep ACT free)
                nc.vector.tensor_copy(raws[:, j, :], psA[:])
                nc.vector.tensor_copy(vt_sb[:, blk], psB[0:64, :])
                nc.vector.tensor_copy(mu_sb[0:2, blk], psB[64:66, :])
                nc.vector.tensor_copy(mu_bf[0:2, blk], psB[64:66, :])
                sq = sqpool.tile([128, 512], bf16)
                nc.vector.tensor_mul(sq[:], raws[:, j, :], raws[:, j, :])
                psSt = psMisc_pool.tile([2, 512], f32, tag="psSt")
                nc.tensor.matmul(psSt[:], sel_sb[:], sq[:], start=True, stop=True)
                nc.vector.tensor_copy(stats_sb[0:2, blk], psSt[:])
                # V transposes for this block: vt rows [64, 128] -> [128, 64]
                for i in range(4):
                    kti = j * 4 + i
                    psv = psMisc_pool.tile([128, 64], bf16, tag="psv")
                    nc.tensor.transpose(psv[:], vt_sb[:, kti * 128:(kti + 1) * 128],
                                        ident_sb[:])
                    nc.vector.tensor_copy(vp[:, kti, 0:64], psv[:])
                if j == 3:
                    # first half fully projected: normalize it while the
                    # second half is still projecting
                    stats_and_normalize(0, 4)
            nc.vector.memset(vp[:, :, 64:65], 1.0)
            stats_and_normalize(4, 8)

          # ---------------- phase 2: scores -> exp -> PV ----------------
          with (
            tc.tile_pool(name="psS", bufs=3, space="PSUM") as psS_pool,
            tc.tile_pool(name="psO", bufs=1, space="PSUM") as psO_pool,
            tc.tile_pool(name="ebuf", bufs=3) as epool,
            tc.tile_pool(name="dn", bufs=1) as dnpool,
          ):
            for qb in range(2):
                qblk = slice(qb * 1024, (qb + 1) * 1024)
                psO = psO_pool.tile([65, 1024], f32)
                for k in range(NKT):
                    psS = psS_pool.tile([128, 1024], f32)
                    for hh in range(2):
                        qs_ = slice(qb * 1024 + hh * 512, qb * 1024 + (hh + 1) * 512)
                        # two concurrent col-group matmuls (M=64 each)
                        nc.tensor.matmul(psS[0:64, hh * 512:(hh + 1) * 512],
                                         kt[:, k * 128:k * 128 + 64], qt[:, qs_],
                                         start=True, stop=True)
                        nc.tensor.matmul(psS[64:128, hh * 512:(hh + 1) * 512],
                                         kt[:, k * 128 + 64:k * 128 + 128], qt[:, qs_],
                                         start=True, stop=True)
                    e = epool.tile([128, 1024], bf16)
                    nc.scalar.activation(e[:], psS[:], EXP, bias=zero_sb[:], scale=0.125)
                    for hh in range(2):
                        nc.tensor.matmul(psO[:, hh * 512:(hh + 1) * 512],
                                         vp[:, k, :], e[:, hh * 512:(hh + 1) * 512],
                                         start=(k == 0), stop=(k == NKT - 1))
                # out and rowsum go to the host; division happens there
                ot = dnpool.tile([65, 1024], f32, tag="ot")
                nc.vector.tensor_copy(ot[:], psO[:])
                nc.gpsimd.dma_start(out=outT_d[:, qblk], in_=ot[:])

    nc.finalize()
    return nc


def _get_nc():
    if "nc" not in _CACHE:
        _CACHE["nc"] = _build_nc()
    return _CACHE["nc"]


def _make_in_maps(xs_q, Wq, Wk, Wv):
    wa32 = np.concatenate([Wq, Wk], axis=1).astype(np.float32)
    wa = wa32.astype(BF16)
    # mu columns from the bf16-rounded weights so the folded identity is tight
    wab = wa.astype(np.float32)
    wmu_q = 8.0 * wab[:, :64].mean(axis=1, keepdims=True)
    wmu_k = -8.0 * wab[:, 64:].mean(axis=1, keepdims=True)
    wb = np.concatenate([Wv.astype(np.float32), wmu_q, wmu_k], axis=1).astype(BF16)
    sel = np.zeros((128, 2), BF16)
    sel[:64, 0] = 1
    sel[64:, 1] = 1
    # row-select lhsTs for the broadcast matmuls (columns = output rows)
    selq = np.zeros((2, 64), BF16); selq[0, :] = 1.0
    selk = np.zeros((2, 64), BF16); selk[1, :] = 1.0
    # mu*rsig pick rows, scaled 1/8; signs chosen so xn = x*rsig - sel(munorm)
    # q: munorm row0 = 8 mu_q rq -> +0.125 ; k: munorm row1 = -8 mu_k rk -> -0.125
    selqm = np.zeros((2, 64), BF16); selqm[0, :] = 0.125
    selkm = np.zeros((2, 64), BF16); selkm[1, :] = -0.125
    ident = np.eye(64, dtype=BF16)
    in_maps = []
    for c in range(8):
        b, h = c // 2, c % 2
        x = xs_q[b]
        q0 = h * HQ
        xr = np.concatenate([x[q0:q0 + HQ], x[:q0], x[q0 + HQ:]], axis=0)
        xst = np.ascontiguousarray(xr.T).astype(BF16)
        in_maps.append({
            "xst": xst, "wa": wa, "wb": wb, "sel": sel,
            "selq": selq, "selk": selk, "selqm": selqm, "selkm": selkm,
            "ident": ident,
        })
    return in_maps


def _ensure_ntff_hook():
    try:
        from antenv.axon_hooks import (
            get_axon_ntff_profile_hook, set_axon_ntff_profile_hook)
        if get_axon_ntff_profile_hook() is None:
            import sys as _sys
            if "/root/.axon_site/trn_agent_boot" not in _sys.path:
                _sys.path.insert(0, "/root/.axon_site/trn_agent_boot")
            import trn_boot
            h = trn_boot._ntff_profile_via_ctypes("/opt/axon/libaxon_pjrt.so")
            if h is not None:
                set_axon_ntff_profile_hook(h)
    except Exception:
        pass


def run(xs_q, Wq, Wk, Wv, trace=False):
    from concourse.bass_utils import run_bass_kernel_spmd
    if trace:
        _ensure_ntff_hook()
    nc = _get_nc()
    in_maps = _make_in_maps(xs_q, Wq, Wk, Wv)
    res = run_bass_kernel_spmd(nc, in_maps, list(range(8)), trace=trace)
    out = np.empty((4, S, H), np.float32)
    for c in range(8):
        b, h = c // 2, c % 2
        r = np.asarray(res.results[c]["outT"])
        out[b, h * HQ:(h + 1) * HQ] = (r[0:64] / r[64:65]).T
    return out, res


def kernel(xs_q, Wq, Wk, Wv):
    out, _ = run(xs_q, Wq, Wk, Wv, trace=False)
    return out

